# revision 1
# baseline (speedup 1.0000x reference)
"""Trainium2 Bass kernel for the KeypointLoss problem.

Full inputs:
  combined_preds [16, 4, 22, 128, 128] f32
  heatmaps       [16, 11, 128, 128]    f32
  labels         [16, 11, 11]          f32
Outputs (matching the reference):
  heat_loss  [16, 4] f32
  label_loss [16, 4] f32

Sharding: pure data parallel over the batch dim — core i handles batches
[2i, 2i+2). Each core computes its [2, 4] slices of both losses.

Per-core algorithm (B_L=2 local batches, S=4 stacks, K=11 keypoints,
H=W=128; G = B_L*S = 8 groups, PL = G*K = 88 planes; all group tiles are
loaded h-major: [h=128 partitions, (k, w) free]):

  heat_loss[b,s] = sum_khw (hm - ht)^2
                 = sum hm^2 - 2*sum hm*ht + sum ht^2      (expansion)
    - sum hm^2, sum ht^2: ScalarE Square with accum_out (per-partition sums)
    - cross term: TensorE bf16 matmuls ht_k^T @ hm_k accumulated over k in
      PSUM (one 512-wide bank per b, one 128-col slice per stack); the
      bank is pre-cleared by a K=1 zero matmul with start=True because
      slice-wise start flags clear the whole bank; only the per-s PSUM
      diagonals are needed (DVE scalar_tensor_tensor with identity + accum)
    - partition sums via ones^T matmul

  label_loss needs per-plane argmax of hm over (h, w):
    - row max R[h, plane] via DVE tensor_reduce(max) over w
    - transpose R on PE -> [plane, h]; M = max_h; x = sum h*(R==M)
    - gather row x of each plane from DRAM via indirect DMA (GPSIMD)
    - y = sum w*(row==M)
    then the small per-plane class/xy/conf losses, masked by validity,
    reduced over k via a single plane->group indicator matmul.

Placement (HW-measured): only 6.5 MB per core is read from HBM (the hm
half of combined_preds plus 9 elements per plane of the lb half). The 10
big loads are split across BOTH HWDGE rings (SP + ACT) — one ring caps
at ~235 GB/s with this 512B-chunk pattern, two measured ~2.6x faster.
Casts: hm groups 0-2 on GPSIMD, hm 3-7 and ht on ACT (GPSIMD casts are
slow on HW and contend with DVE for the shared SBUF port). ht^2 and the
hm^2 of groups 0-2 ride PE (diag of bf16 self-matmuls) to relieve ACT,
the busiest engine. DVE keeps rowmaxes, PSUM diagonals (deferred past
the argmax head via explicit dep edges), and the small tail ops.
Measured on-device span ~30us/iteration (For_i loop method).
"""

import os as _os
import sys

for _p in ("/opt/trn_rl_repo", "/root/.axon_site/_ro/trn_rl_repo"):
    if _p not in sys.path:
        sys.path.append(_p)

from contextlib import ExitStack

import numpy as np

# Problem constants (hardcoded per the task contract).
B, S, K, H, W = 16, 4, 11, 128, 128
NCORES = 8
BL = B // NCORES          # local batch per core = 2
G = BL * S                # groups per core = 8
PL = G * K                # planes per core = 88
KW = K * W                # free size of one group tile = 1408
C2 = 2 * K                # channel count of combined_preds = 22

_CACHE = {}
_SKIP = set()  # ablation flags for bench experiments


def _build_module(reps=1, loop_n=1):
    import concourse.bass as bass
    import concourse.tile as tile
    from concourse import bacc, mybir

    f32 = mybir.dt.float32
    bf16 = mybir.dt.bfloat16
    Alu = mybir.AluOpType
    Act = mybir.ActivationFunctionType
    Ax = mybir.AxisListType

    nc = bacc.Bacc("TRN2", debug=False, enable_asserts=False, num_devices=1)

    cp = nc.dram_tensor("cp", [BL, S, C2, H, W], f32, kind="ExternalInput").ap()
    hmr = nc.dram_tensor("hmr", [BL, K, H, W], f32, kind="ExternalInput").ap()
    lbl = nc.dram_tensor("lbl", [BL, K, 11], f32, kind="ExternalInput").ap()
    out_heat = nc.dram_tensor("out_heat", [1, G], f32, kind="ExternalOutput").ap()
    out_label = nc.dram_tensor("out_label", [1, G], f32, kind="ExternalOutput").ap()

    # Inline constants packed into ONE tensor -> one aux DMA.
    # cols [0,128) identity, [128,256) iota, 256 ones, 257 rbase, [258,266) kmap
    aux_np = np.zeros((128, 266), np.float32)
    aux_np[:, 0:128] = np.eye(128, dtype=np.float32)
    aux_np[:, 128:256] = np.arange(128, dtype=np.float32)[None, :]
    aux_np[:, 256] = 1.0
    # rbase: DRAM row index (in units of W-element rows) of (plane, h=0)
    # within cp viewed as [(BL*S*C2*H), W].
    for g in range(G):
        b, s = divmod(g, S)
        for k in range(K):
            aux_np[g * K + k, 257] = ((b * S + s) * C2 + k) * H
            aux_np[g * K + k, 258 + g] = 1.0  # plane->group indicator
    aux_c = nc.inline_tensor(aux_np, "auxc").ap()

    with tile.TileContext(nc) as tc, ExitStack() as ctx:
        bufs = 1 if reps == 1 else 2
        sb = ctx.enter_context(tc.tile_pool(name="sb", bufs=bufs))
        scr = ctx.enter_context(tc.tile_pool(name="scr", bufs=2))
        ps = ctx.enter_context(tc.tile_pool(name="ps", bufs=1, space="PSUM"))

        def emit():
            # aux constants first — one small DMA; id_t gates the PSUM diag
            # extraction, so it must not sit behind the big loads.
            aux_t = sb.tile([128, 266], f32, name="aux_t")
            nc.sync.dma_start(aux_t[:], aux_c)
            id_t = aux_t[:, 0:128]
            io_t = aux_t[:, 128:256]
            on_t = aux_t[:, 256:257]
            rb_t = aux_t[0:PL, 257:258]
            km_t = aux_t[0:PL, 258:266]

            # ---- big loads split across BOTH HWDGE rings (SP + ACT): one ring
            # alone caps at ~235 GB/s with this 512B-chunk pattern; two rings
            # measured ~2.6x faster. Each ring is FIFO, so bigs go first. ----
            ht_ts = []
            for b in range(BL):
                ht_t = sb.tile([128, KW], f32, name=f"ht{b}")
                eng = nc.sync if b == 0 else nc.scalar
                eng.dma_start(
                    ht_t[:].rearrange("h (k w) -> h k w", k=K),
                    hmr[b].rearrange("k h w -> h k w"),
                )
                ht_ts.append(ht_t)
            hm_big = sb.tile([128, G * KW], f32, name="hm_big")

            def hm_g(g):
                return hm_big[:, g * KW : (g + 1) * KW]

            for g in range(G):
                b, s = divmod(g, S)
                eng = nc.sync if g % 2 == 0 else nc.scalar
                eng.dma_start(
                    hm_g(g).rearrange("h (k w) -> h k w", k=K),
                    cp[b, s, 0:K].rearrange("k h w -> h k w"),
                )

            # ---- small loads (same ring, after the big ones) ----
            APc = type(lbl)
            lblr = sb.tile([PL, 11], f32, name="lblr")
            for b in range(BL):
                src_b = APc(lbl.tensor, b * K * 11, [[0, S], [11, K], [1, 11]])
                nc.sync.dma_start(lblr[b * S * K : (b + 1) * S * K, :], src_b)
            pred9 = sb.tile([PL, 9], f32, name="pred9")
            nc.sync.dma_start(pred9[:], cp[:, :, K:C2, 0, 0:9])

            # zero row for the K=1 bank-clearing matmuls
            zrow = sb.tile([1, 512], bf16, name="zrow")
            nc.gpsimd.memset(zrow[:], 0.0)

            # ---- bf16 casts for the cross-term matmuls ----
            # ht casts ride DVE's early idle (fastest caster, data arrives
            # first); hm groups 0..5 ride GPSIMD; hm6/hm7 ride ACT (below).
            htb_ts = []
            _htc = "act"
            _ht0 = _os.environ.get("HT0", "act")
            for b in range(BL):
                htb = sb.tile([128, KW], bf16, name=f"htb{b}")
                if b == 0 and _ht0 == "pool":
                    # ht0 lands first; casting it on GPSIMD fills the idle
                    # window before the first hm group arrives
                    nc.gpsimd.tensor_copy(htb[:], ht_ts[b][:])
                elif _htc == "act":
                    nc.scalar.copy(htb[:], ht_ts[b][:])
                elif _htc == "pool":
                    nc.gpsimd.tensor_copy(htb[:], ht_ts[b][:])
                else:
                    nc.vector.tensor_copy(htb[:], ht_ts[b][:])
                htb_ts.append(htb)
            hmb16 = sb.tile([128, G * KW], bf16, name="hmb16")
            _ncp = 3
            for g in range(_ncp):
                if "cast" in _SKIP:
                    nc.gpsimd.memset(hmb16[:, g * KW : (g + 1) * KW], 0.0)
                    break
                nc.gpsimd.tensor_copy(hmb16[:, g * KW : (g + 1) * KW], hm_g(g))

            # ---- ht squares early on ACT ----
            acc_hm = sb.tile([128, G], f32, name="acc_hm")
            acc_ht = sb.tile([128, BL], f32, name="acc_ht")
            diag = sb.tile([128, G], f32, name="diag")
            R_all = sb.tile([128, PL], f32, name="R_all")

            # ---- per-group: rowmax (DVE) + sum hm^2 (ACT) ----
            if "rowmax" in _SKIP:
                nc.vector.memset(R_all[:], 0.0)
            if "sq" in _SKIP:
                nc.vector.memset(acc_hm[:], 0.0)
            for g in range(G):
                if "rowmax" in _SKIP:
                    break
                nc.vector.tensor_reduce(
                    out=R_all[:, g * K : (g + 1) * K],
                    in_=hm_g(g).rearrange("h (k w) -> h k w", k=K),
                    axis=Ax.X,
                    op=Alu.max,
                )
                if g >= _ncp:
                    # later groups' bf16 casts ride ACT, ahead of their
                    # squares, so the late cross matmuls aren't gated on the
                    # serial GPSIMD cast queue
                    nc.scalar.copy(hmb16[:, g * KW : (g + 1) * KW], hm_g(g))
                if "sq" not in _SKIP and g >= 3:
                    sq = scr.tile([128, KW], f32, name=f"sq{g}", tag="sq")
                    nc.scalar.activation(
                        out=sq[:], in_=hm_g(g), func=Act.Square,
                        accum_out=acc_hm[:, g : g + 1],
                    )

            # ---- cross-term matmuls (PE, bf16, N=128 each, accumulated over k
            # into one pre-cleared 512-wide PSUM bank per b) ----
            psx = {}
            for b in range(BL):
                psx[b] = ps.tile([128, 512], f32, name=f"psx{b}", tag=f"pxb{b}")
                nc.tensor.matmul(
                    out=psx[b][:], lhsT=zrow[:, 0:128], rhs=zrow[:],
                    start=True, stop=False, skip_group_check=True,
                )

            def cross_mms(b, ss):
                for k in range(K):
                    for s in ss:
                        g = b * S + s
                        nc.tensor.matmul(
                            out=psx[b][:, s * 128 : (s + 1) * 128],
                            lhsT=htb_ts[b][:, k * 128 : (k + 1) * 128],
                            rhs=hmb16[:, g * KW + k * 128 : g * KW + (k + 1) * 128],
                            start=False,
                            stop=(k == K - 1),
                            skip_group_check=True,
                        )

            # sum ht^2 via PE as well (diag of htb^T htb) — frees ACT time
            psum_ht = ps.tile([128, 256], f32, name="psum_ht", tag="pht")
            nc.tensor.matmul(
                out=psum_ht[:], lhsT=zrow[:, 0:128], rhs=zrow[:, 0:256],
                start=True, stop=False, skip_group_check=True,
            )
            for b in range(BL):
                for k in range(K):
                    nc.tensor.matmul(
                        out=psum_ht[:, b * 128 : (b + 1) * 128],
                        lhsT=htb_ts[b][:, k * 128 : (k + 1) * 128],
                        rhs=htb_ts[b][:, k * 128 : (k + 1) * 128],
                        start=False,
                        stop=(k == K - 1),
                        skip_group_check=True,
                    )
            # sum hm^2 for groups 0/1 via PE too (their bf16 casts are on
            # GPSIMD and land first) — relieves ScalarE, the busiest engine
            psum_sq = ps.tile([128, 384], f32, name="psum_sq", tag="psq")
            nc.tensor.matmul(
                out=psum_sq[:], lhsT=zrow[:, 0:128], rhs=zrow[:, 0:384],
                start=True, stop=False, skip_group_check=True,
            )
            for g01 in range(3):
                for k in range(K):
                    sl = slice(g01 * KW + k * 128, g01 * KW + (k + 1) * 128)
                    nc.tensor.matmul(
                        out=psum_sq[:, g01 * 128 : (g01 + 1) * 128],
                        lhsT=hmb16[:, sl],
                        rhs=hmb16[:, sl],
                        start=False,
                        stop=(k == K - 1),
                        skip_group_check=True,
                    )

            cross_mms(0, [0, 1, 2, 3])
            cross_mms(1, [0, 1, 2])

            # ---- argmax head: transpose R, global max, x, gather issue ----
            psum_rt = ps.tile([PL, 128], f32, name="psum_rt", tag="rt")
            nc.tensor.transpose(out=psum_rt[:], in_=R_all[:], identity=id_t[:])
            Mv = sb.tile([PL, 1], f32, name="Mv")
            nc.vector.tensor_reduce(out=Mv[:], in_=psum_rt[:], axis=Ax.X, op=Alu.max)
            xsc = scr.tile([PL, 128], f32, name="xsc", tag="xysc")
            xf = sb.tile([PL, 1], f32, name="xf")
            nc.vector.scalar_tensor_tensor(
                out=xsc[:], in0=psum_rt[:], scalar=Mv[:, 0:1], in1=io_t[0:PL, :],
                op0=Alu.is_equal, op1=Alu.mult, accum_out=xf[:],
            )
            ridu = sb.tile([PL, 1], mybir.dt.uint32, name="ridu")
            ridu_inst = nc.vector.tensor_tensor(
                out=ridu[:], in0=xf[:], in1=rb_t[:], op=Alu.add
            )
            gath = sb.tile([PL, 128], f32, name="gath")
            nc.gpsimd.indirect_dma_start(
                out=gath[:],
                out_offset=None,
                in_=cp.rearrange("b s c h w -> (b s c h) w"),
                in_offset=bass.IndirectOffsetOnAxis(ap=ridu[:, 0:1], axis=0),
            )

            # ---- remaining cross matmuls + all diagonals (fill the gather gap)
            cross_mms(1, [3])
            for b in range(BL):
                for s in range(S):
                    g = b * S + s
                    dsc = scr.tile([128, 128], f32, name=f"dsc{g}", tag="dsc")
                    d_inst = nc.vector.scalar_tensor_tensor(
                        out=dsc[:],
                        in0=psx[b][:, s * 128 : (s + 1) * 128],
                        scalar=1.0,
                        in1=id_t[:],
                        op0=Alu.bypass,
                        op1=Alu.mult,
                        accum_out=diag[:, g : g + 1],
                    )
                    # keep the diagonals out of the rowmax/argmax head: they can
                    # fill the gather window instead
                    tile.add_dep_helper(
                        d_inst.ins, ridu_inst.ins, sync=False,
                        reason="defer diag past argmax head",
                    )

            for g01 in range(3):
                dsq = scr.tile([128, 128], f32, name=f"dsq{g01}", tag="dsc")
                dq_inst = nc.vector.scalar_tensor_tensor(
                    out=dsq[:],
                    in0=psum_sq[:, g01 * 128 : (g01 + 1) * 128],
                    scalar=1.0,
                    in1=id_t[:],
                    op0=Alu.bypass,
                    op1=Alu.mult,
                    accum_out=acc_hm[:, g01 : g01 + 1],
                )
                tile.add_dep_helper(
                    dq_inst.ins, ridu_inst.ins, sync=False,
                    reason="defer hm2 diag past argmax head",
                )
            for b in range(BL):
                dht = scr.tile([128, 128], f32, name=f"dht{b}", tag="dsc")
                dh_inst = nc.vector.scalar_tensor_tensor(
                    out=dht[:],
                    in0=psum_ht[:, b * 128 : (b + 1) * 128],
                    scalar=1.0,
                    in1=id_t[:],
                    op0=Alu.bypass,
                    op1=Alu.mult,
                    accum_out=acc_ht[:, b : b + 1],
                )
                tile.add_dep_helper(
                    dh_inst.ins, ridu_inst.ins, sync=False,
                    reason="defer ht diag past argmax head",
                )

            # ---- y from the gathered rows ----
            ysc = scr.tile([PL, 128], f32, name="ysc", tag="xysc")
            yf = sb.tile([PL, 1], f32, name="yf")
            nc.vector.scalar_tensor_tensor(
                out=ysc[:], in0=gath[:], scalar=Mv[:, 0:1], in1=io_t[0:PL, :],
                op0=Alu.is_equal, op1=Alu.mult, accum_out=yf[:],
            )

            # ---- heat loss: combine per-partition pieces, partition-sum, out ----
            ucomb = sb.tile([128, G], f32, name="ucomb")
            for b in range(BL):
                nc.vector.tensor_tensor(
                    out=ucomb[:, b * S : (b + 1) * S],
                    in0=acc_hm[:, b * S : (b + 1) * S],
                    in1=acc_ht[:, b : b + 1].to_broadcast([128, S]),
                    op=Alu.add,
                )
            acc_fin = sb.tile([128, G], f32, name="acc_fin")
            nc.vector.scalar_tensor_tensor(
                out=acc_fin[:], in0=diag[:], scalar=-2.0, in1=ucomb[:],
                op0=Alu.mult, op1=Alu.add,
            )
            psum_hs = ps.tile([1, G], f32, name="psum_hs", tag="fin")
            nc.tensor.matmul(
                out=psum_hs[:], lhsT=on_t[:], rhs=acc_fin[:], start=True, stop=True
            )
            heat_row = sb.tile([1, G], f32, name="heat_row")
            nc.vector.tensor_copy(out=heat_row[:], in_=psum_hs[:])
            nc.sync.dma_start(out_heat, heat_row[:])

            # ---- label loss ----
            cdiff = sb.tile([PL, 7], f32, name="cdiff")
            nc.vector.tensor_tensor(
                out=cdiff[:], in0=pred9[:, 0:7], in1=lblr[:, 0:7], op=Alu.subtract
            )
            csc = sb.tile([PL, 7], f32, name="csc")
            cls = sb.tile([PL, 1], f32, name="cls")
            nc.scalar.activation(
                out=csc[:], in_=cdiff[:], func=Act.Square, accum_out=cls[:]
            )
            conf = sb.tile([PL, 1], f32, name="conf")
            nc.scalar.activation(
                out=conf[:], in_=Mv[:], func=Act.Square, bias=1.0, scale=-1.0
            )
            t1 = sb.tile([PL, 1], f32, name="t1")
            nc.vector.tensor_tensor(t1[:], lblr[:, 9:10], lblr[:, 7:8], Alu.add)
            t3 = sb.tile([PL, 1], f32, name="t3")
            nc.vector.tensor_tensor(t3[:], lblr[:, 10:11], lblr[:, 8:9], Alu.add)
            gmin = sb.tile([PL, 1], f32, name="gmin")
            nc.vector.tensor_tensor(gmin[:], lblr[:, 9:10], lblr[:, 10:11], Alu.min)
            gmax = sb.tile([PL, 1], f32, name="gmax")
            nc.vector.tensor_tensor(gmax[:], lblr[:, 9:10], lblr[:, 10:11], Alu.max)
            c1 = sb.tile([PL, 1], f32, name="c1")
            nc.vector.tensor_scalar(c1[:], gmin[:], 0.0, None, Alu.is_gt)
            c2t = sb.tile([PL, 1], f32, name="c2t")
            nc.vector.tensor_scalar(c2t[:], gmax[:], float(H), None, Alu.is_lt)
            vv = sb.tile([PL, 1], f32, name="vv")
            nc.vector.tensor_tensor(vv[:], c1[:], c2t[:], Alu.mult)

            t2 = sb.tile([PL, 1], f32, name="t2")
            nc.vector.tensor_tensor(t2[:], xf[:], pred9[:, 7:8], Alu.add)
            tx = sb.tile([PL, 1], f32, name="tx")
            nc.vector.tensor_tensor(tx[:], t1[:], t2[:], Alu.subtract)
            txs = sb.tile([PL, 1], f32, name="txs")
            nc.vector.tensor_tensor(txs[:], tx[:], tx[:], Alu.mult)
            t4 = sb.tile([PL, 1], f32, name="t4")
            nc.vector.tensor_tensor(t4[:], yf[:], pred9[:, 8:9], Alu.add)
            ty = sb.tile([PL, 1], f32, name="ty")
            nc.vector.tensor_tensor(ty[:], t3[:], t4[:], Alu.subtract)
            xyl = sb.tile([PL, 1], f32, name="xyl")
            nc.vector.scalar_tensor_tensor(
                out=xyl[:], in0=ty[:], scalar=ty[:, 0:1], in1=txs[:],
                op0=Alu.mult, op1=Alu.add,
            )
            tot = sb.tile([PL, 1], f32, name="tot")
            nc.vector.tensor_tensor(tot[:], cls[:], xyl[:], Alu.add)
            tot2 = sb.tile([PL, 1], f32, name="tot2")
            nc.vector.tensor_tensor(tot2[:], tot[:], conf[:], Alu.add)
            perkp = sb.tile([PL, 1], f32, name="perkp")
            nc.vector.tensor_tensor(perkp[:], tot2[:], vv[:], Alu.mult)
            psum_lk = ps.tile([1, G], f32, name="psum_lk", tag="fin2")
            nc.tensor.matmul(
                out=psum_lk[:], lhsT=perkp[:], rhs=km_t[:], start=True, stop=True
            )
            lab_row = sb.tile([1, G], f32, name="lab_row")
            nc.vector.tensor_copy(out=lab_row[:], in_=psum_lk[:])
            nc.sync.dma_start(out_label, lab_row[:])


        if loop_n > 1:
            # on-device timing loop: each iteration is separated by the
            # For_i back-edge barrier, so wall time ~= N * (span + ~2us)
            with tc.For_i(0, loop_n, 1):
                emit()
        else:
            for _ in range(reps):
                emit()

    nc.compile()
    return nc


def _get_nc(reps=1, loop_n=1):
    key = f"nc{reps}_{loop_n}"
    if key not in _CACHE:
        _CACHE[key] = _build_module(reps, loop_n)
    return _CACHE[key]


def _in_maps(combined_preds, heatmaps, labels):
    cp = np.ascontiguousarray(combined_preds, dtype=np.float32)
    hmr = np.ascontiguousarray(heatmaps, dtype=np.float32)
    lb = np.ascontiguousarray(labels, dtype=np.float32)
    maps = []
    for i in range(NCORES):
        b0 = BL * i
        maps.append(
            {
                "cp": np.ascontiguousarray(cp[b0 : b0 + BL]),
                "hmr": np.ascontiguousarray(hmr[b0 : b0 + BL]),
                "lbl": np.ascontiguousarray(lb[b0 : b0 + BL]),
            }
        )
    return maps


def run(combined_preds, heatmaps, labels, trace=False):
    """Run on hardware; returns ((heat, label), BassKernelResults)."""
    from concourse import bass_utils

    nc = _get_nc()
    res = bass_utils.run_bass_kernel_spmd(
        nc,
        _in_maps(combined_preds, heatmaps, labels),
        core_ids=list(range(NCORES)),
        trace=trace,
    )
    heat = np.concatenate(
        [res.results[i]["out_heat"].reshape(BL, S) for i in range(NCORES)], axis=0
    )
    lab = np.concatenate(
        [res.results[i]["out_label"].reshape(BL, S) for i in range(NCORES)], axis=0
    )
    return (heat, lab), res


def kernel(combined_preds, heatmaps, labels):
    (heat, lab), _ = run(combined_preds, heatmaps, labels)
    return heat, lab



# revision 2
# speedup vs baseline: 1.7122x; 1.7122x over previous
"""Trainium2 Bass kernel for the KeypointLoss problem.

Full inputs:
  combined_preds [16, 4, 22, 128, 128] f32
  heatmaps       [16, 11, 128, 128]    f32
  labels         [16, 11, 11]          f32
Outputs (matching the reference):
  heat_loss  [16, 4] f32
  label_loss [16, 4] f32

Sharding: pure data parallel over the batch dim — core i handles batches
[2i, 2i+2). Each core computes its [2, 4] slices of both losses.

Per-core algorithm (B_L=2 local batches, S=4 stacks, K=11 keypoints,
H=W=128; G = B_L*S = 8 groups, PL = G*K = 88 planes; all group tiles are
loaded h-major: [h=128 partitions, (k, w) free]):

  heat_loss[b,s] = sum_khw (hm - ht)^2
                 = sum hm^2 - 2*sum hm*ht + sum ht^2      (expansion)
    - sum hm^2, sum ht^2: ScalarE Square with accum_out (per-partition sums)
    - cross term: TensorE bf16 matmuls ht_k^T @ hm_k accumulated over k in
      PSUM (one 512-wide bank per b, one 128-col slice per stack); the
      bank is pre-cleared by a K=1 zero matmul with start=True because
      slice-wise start flags clear the whole bank; only the per-s PSUM
      diagonals are needed (DVE scalar_tensor_tensor with identity + accum)
    - partition sums via ones^T matmul

  label_loss needs per-plane argmax of hm over (h, w):
    - row max R[h, plane] via DVE tensor_reduce(max) over w
    - transpose R on PE -> [plane, h]; M = max_h; x = sum h*(R==M)
    - gather row x of each plane from DRAM via indirect DMA (GPSIMD)
    - y = sum w*(row==M)
    then the small per-plane class/xy/conf losses, masked by validity,
    reduced over k via a single plane->group indicator matmul.

Placement (HW-measured): only 6.5 MB per core is read from HBM (the hm
half of combined_preds plus 9 elements per plane of the lb half). The 10
big loads are split across BOTH HWDGE rings (SP + ACT) — one ring caps
at ~235 GB/s with this 512B-chunk pattern, two measured ~2.6x faster.
Casts: hm groups 0-2 on GPSIMD, hm 3-7 and ht on ACT (GPSIMD casts are
slow on HW and contend with DVE for the shared SBUF port). ht^2 and the
hm^2 of groups 0-2 ride PE (diag of bf16 self-matmuls) to relieve ACT,
the busiest engine. DVE keeps rowmaxes, PSUM diagonals (deferred past
the argmax head via explicit dep edges), and the small tail ops.

Tail (vs the earlier revision): the label arithmetic is fused into five
scalar_tensor_tensor ops (u=(x+p7)-t1, w1=u*u+cls, v=(y+p8)-t3,
w2=v*v+w1, perkp=(w2+conf)*vv — signs fold away under the squares), and
both final reductions land in ONE PSUM bank (heat matmul start=True /
label matmul stop=True) so a single copy + a single [1,16] output DMA
replace the two copy+DMA pairs.  Measured ~37.6us/iteration vs ~42us for
the previous revision in the same session (For_i loop method).
"""

import os as _os
import sys

for _p in ("/opt/trn_rl_repo", "/root/.axon_site/_ro/trn_rl_repo"):
    if _p not in sys.path:
        sys.path.append(_p)

from contextlib import ExitStack

import numpy as np

# Problem constants (hardcoded per the task contract).
B, S, K, H, W = 16, 4, 11, 128, 128
NCORES = 8
BL = B // NCORES          # local batch per core = 2
G = BL * S                # groups per core = 8
PL = G * K                # planes per core = 88
KW = K * W                # free size of one group tile = 1408
C2 = 2 * K                # channel count of combined_preds = 22

_CACHE = {}
_SKIP = set()  # ablation flags for bench experiments


def _build_module(reps=1, loop_n=1):
    import concourse.bass as bass
    import concourse.tile as tile
    from concourse import bacc, mybir

    f32 = mybir.dt.float32
    bf16 = mybir.dt.bfloat16
    Alu = mybir.AluOpType
    Act = mybir.ActivationFunctionType
    Ax = mybir.AxisListType

    nc = bacc.Bacc("TRN2", debug=False, enable_asserts=False, num_devices=1)

    cp = nc.dram_tensor("cp", [BL, S, C2, H, W], f32, kind="ExternalInput").ap()
    hmr = nc.dram_tensor("hmr", [BL, K, H, W], f32, kind="ExternalInput").ap()
    lbl = nc.dram_tensor("lbl", [BL, K, 11], f32, kind="ExternalInput").ap()
    out_all = nc.dram_tensor("out_all", [1, 2 * G], f32, kind="ExternalOutput").ap()

    # Inline constants packed into ONE tensor -> one aux DMA.
    # cols [0,128) identity, [128,256) iota, 256 ones, 257 rbase, [258,266) kmap
    aux_np = np.zeros((128, 266), np.float32)
    aux_np[:, 0:128] = np.eye(128, dtype=np.float32)
    aux_np[:, 128:256] = np.arange(128, dtype=np.float32)[None, :]
    aux_np[:, 256] = 1.0
    # rbase: DRAM row index (in units of W-element rows) of (plane, h=0)
    # within cp viewed as [(BL*S*C2*H), W].
    for g in range(G):
        b, s = divmod(g, S)
        for k in range(K):
            aux_np[g * K + k, 257] = ((b * S + s) * C2 + k) * H
            aux_np[g * K + k, 258 + g] = 1.0  # plane->group indicator
    aux_c = nc.inline_tensor(aux_np, "auxc").ap()

    with tile.TileContext(nc) as tc, ExitStack() as ctx:
        bufs = 1 if reps == 1 else 2
        sb = ctx.enter_context(tc.tile_pool(name="sb", bufs=bufs))
        scr = ctx.enter_context(tc.tile_pool(name="scr", bufs=2))
        ps = ctx.enter_context(tc.tile_pool(name="ps", bufs=1, space="PSUM"))

        def emit():
            # aux constants first — one small DMA; id_t gates the PSUM diag
            # extraction, so it must not sit behind the big loads.
            aux_t = sb.tile([128, 266], f32, name="aux_t")
            nc.sync.dma_start(aux_t[:], aux_c)
            id_t = aux_t[:, 0:128]
            io_t = aux_t[:, 128:256]
            on_t = aux_t[:, 256:257]
            rb_t = aux_t[0:PL, 257:258]
            km_t = aux_t[0:PL, 258:266]

            # ---- big loads split across BOTH HWDGE rings (SP + ACT): one ring
            # alone caps at ~235 GB/s with this 512B-chunk pattern; two rings
            # measured ~2.6x faster. Each ring is FIFO, so bigs go first. ----
            ht_ts = []
            for b in range(BL):
                ht_t = sb.tile([128, KW], f32, name=f"ht{b}")
                eng = nc.sync if b == 0 else nc.scalar
                eng.dma_start(
                    ht_t[:].rearrange("h (k w) -> h k w", k=K),
                    hmr[b].rearrange("k h w -> h k w"),
                )
                ht_ts.append(ht_t)
            hm_big = sb.tile([128, G * KW], f32, name="hm_big")

            def hm_g(g):
                return hm_big[:, g * KW : (g + 1) * KW]

            for g in range(G):
                b, s = divmod(g, S)
                eng = nc.sync if g % 2 == 0 else nc.scalar
                eng.dma_start(
                    hm_g(g).rearrange("h (k w) -> h k w", k=K),
                    cp[b, s, 0:K].rearrange("k h w -> h k w"),
                )

            # ---- small loads (same ring, after the big ones) ----
            APc = type(lbl)
            lblr = sb.tile([PL, 11], f32, name="lblr")
            for b in range(BL):
                src_b = APc(lbl.tensor, b * K * 11, [[0, S], [11, K], [1, 11]])
                nc.sync.dma_start(lblr[b * S * K : (b + 1) * S * K, :], src_b)
            pred9 = sb.tile([PL, 9], f32, name="pred9")
            nc.sync.dma_start(pred9[:], cp[:, :, K:C2, 0, 0:9])

            # zero row for the K=1 bank-clearing matmuls
            zrow = sb.tile([1, 512], bf16, name="zrow")
            nc.gpsimd.memset(zrow[:], 0.0)

            # ---- bf16 casts for the cross-term matmuls ----
            # ht casts ride DVE's early idle (fastest caster, data arrives
            # first); hm groups 0..5 ride GPSIMD; hm6/hm7 ride ACT (below).
            htb_ts = []
            _htc = "act"
            _ht0 = _os.environ.get("HT0", "act")
            for b in range(BL):
                htb = sb.tile([128, KW], bf16, name=f"htb{b}")
                if b == 0 and _ht0 == "pool":
                    # ht0 lands first; casting it on GPSIMD fills the idle
                    # window before the first hm group arrives
                    nc.gpsimd.tensor_copy(htb[:], ht_ts[b][:])
                elif _htc == "act":
                    nc.scalar.copy(htb[:], ht_ts[b][:])
                elif _htc == "pool":
                    nc.gpsimd.tensor_copy(htb[:], ht_ts[b][:])
                else:
                    nc.vector.tensor_copy(htb[:], ht_ts[b][:])
                htb_ts.append(htb)
            hmb16 = sb.tile([128, G * KW], bf16, name="hmb16")
            _ncp = 3
            for g in range(_ncp):
                if "cast" in _SKIP:
                    nc.gpsimd.memset(hmb16[:, g * KW : (g + 1) * KW], 0.0)
                    break
                nc.gpsimd.tensor_copy(hmb16[:, g * KW : (g + 1) * KW], hm_g(g))

            # ---- ht squares early on ACT ----
            acc_hm = sb.tile([128, G], f32, name="acc_hm")
            acc_ht = sb.tile([128, BL], f32, name="acc_ht")
            diag = sb.tile([128, G], f32, name="diag")
            R_all = sb.tile([128, PL], f32, name="R_all")

            # ---- per-group: rowmax (DVE) + sum hm^2 (ACT) ----
            if "rowmax" in _SKIP:
                nc.vector.memset(R_all[:], 0.0)
            if "sq" in _SKIP:
                nc.vector.memset(acc_hm[:], 0.0)
            for g in range(G):
                if "rowmax" in _SKIP:
                    break
                nc.vector.tensor_reduce(
                    out=R_all[:, g * K : (g + 1) * K],
                    in_=hm_g(g).rearrange("h (k w) -> h k w", k=K),
                    axis=Ax.X,
                    op=Alu.max,
                )
                if g >= _ncp:
                    # later groups' bf16 casts ride ACT, ahead of their
                    # squares, so the late cross matmuls aren't gated on the
                    # serial GPSIMD cast queue
                    nc.scalar.copy(hmb16[:, g * KW : (g + 1) * KW], hm_g(g))
                if "sq" not in _SKIP and g >= 3:
                    sq = scr.tile([128, KW], f32, name=f"sq{g}", tag="sq")
                    nc.scalar.activation(
                        out=sq[:], in_=hm_g(g), func=Act.Square,
                        accum_out=acc_hm[:, g : g + 1],
                    )

            # ---- cross-term matmuls (PE, bf16, N=128 each, accumulated over k
            # into one pre-cleared 512-wide PSUM bank per b) ----
            psx = {}
            for b in range(BL):
                psx[b] = ps.tile([128, 512], f32, name=f"psx{b}", tag=f"pxb{b}")
                nc.tensor.matmul(
                    out=psx[b][:], lhsT=zrow[:, 0:128], rhs=zrow[:],
                    start=True, stop=False, skip_group_check=True,
                )

            def cross_mms(b, ss):
                for k in range(K):
                    for s in ss:
                        g = b * S + s
                        nc.tensor.matmul(
                            out=psx[b][:, s * 128 : (s + 1) * 128],
                            lhsT=htb_ts[b][:, k * 128 : (k + 1) * 128],
                            rhs=hmb16[:, g * KW + k * 128 : g * KW + (k + 1) * 128],
                            start=False,
                            stop=(k == K - 1),
                            skip_group_check=True,
                        )

            # sum ht^2 via PE as well (diag of htb^T htb) — frees ACT time
            psum_ht = ps.tile([128, 256], f32, name="psum_ht", tag="pht")
            nc.tensor.matmul(
                out=psum_ht[:], lhsT=zrow[:, 0:128], rhs=zrow[:, 0:256],
                start=True, stop=False, skip_group_check=True,
            )
            for b in range(BL):
                for k in range(K):
                    nc.tensor.matmul(
                        out=psum_ht[:, b * 128 : (b + 1) * 128],
                        lhsT=htb_ts[b][:, k * 128 : (k + 1) * 128],
                        rhs=htb_ts[b][:, k * 128 : (k + 1) * 128],
                        start=False,
                        stop=(k == K - 1),
                        skip_group_check=True,
                    )
            # sum hm^2 for groups 0/1 via PE too (their bf16 casts are on
            # GPSIMD and land first) — relieves ScalarE, the busiest engine
            psum_sq = ps.tile([128, 384], f32, name="psum_sq", tag="psq")
            nc.tensor.matmul(
                out=psum_sq[:], lhsT=zrow[:, 0:128], rhs=zrow[:, 0:384],
                start=True, stop=False, skip_group_check=True,
            )
            for g01 in range(3):
                for k in range(K):
                    sl = slice(g01 * KW + k * 128, g01 * KW + (k + 1) * 128)
                    nc.tensor.matmul(
                        out=psum_sq[:, g01 * 128 : (g01 + 1) * 128],
                        lhsT=hmb16[:, sl],
                        rhs=hmb16[:, sl],
                        start=False,
                        stop=(k == K - 1),
                        skip_group_check=True,
                    )

            cross_mms(0, [0, 1, 2, 3])
            cross_mms(1, [0, 1, 2])

            # ---- argmax head: transpose R, global max, x, gather issue ----
            psum_rt = ps.tile([PL, 128], f32, name="psum_rt", tag="rt")
            nc.tensor.transpose(out=psum_rt[:], in_=R_all[:], identity=id_t[:])
            Mv = sb.tile([PL, 1], f32, name="Mv")
            nc.vector.tensor_reduce(out=Mv[:], in_=psum_rt[:], axis=Ax.X, op=Alu.max)
            xsc = scr.tile([PL, 128], f32, name="xsc", tag="xysc")
            xf = sb.tile([PL, 1], f32, name="xf")
            nc.vector.scalar_tensor_tensor(
                out=xsc[:], in0=psum_rt[:], scalar=Mv[:, 0:1], in1=io_t[0:PL, :],
                op0=Alu.is_equal, op1=Alu.mult, accum_out=xf[:],
            )
            ridu = sb.tile([PL, 1], mybir.dt.uint32, name="ridu")
            ridu_inst = nc.vector.tensor_tensor(
                out=ridu[:], in0=xf[:], in1=rb_t[:], op=Alu.add
            )
            gath = sb.tile([PL, 128], f32, name="gath")
            nc.gpsimd.indirect_dma_start(
                out=gath[:],
                out_offset=None,
                in_=cp.rearrange("b s c h w -> (b s c h) w"),
                in_offset=bass.IndirectOffsetOnAxis(ap=ridu[:, 0:1], axis=0),
            )

            # ---- remaining cross matmuls + all diagonals (fill the gather gap)
            cross_mms(1, [3])
            for b in range(BL):
                for s in range(S):
                    g = b * S + s
                    dsc = scr.tile([128, 128], f32, name=f"dsc{g}", tag="dsc")
                    d_inst = nc.vector.scalar_tensor_tensor(
                        out=dsc[:],
                        in0=psx[b][:, s * 128 : (s + 1) * 128],
                        scalar=1.0,
                        in1=id_t[:],
                        op0=Alu.bypass,
                        op1=Alu.mult,
                        accum_out=diag[:, g : g + 1],
                    )
                    # keep the diagonals out of the rowmax/argmax head: they can
                    # fill the gather window instead
                    tile.add_dep_helper(
                        d_inst.ins, ridu_inst.ins, sync=False,
                        reason="defer diag past argmax head",
                    )

            for g01 in range(3):
                dsq = scr.tile([128, 128], f32, name=f"dsq{g01}", tag="dsc")
                dq_inst = nc.vector.scalar_tensor_tensor(
                    out=dsq[:],
                    in0=psum_sq[:, g01 * 128 : (g01 + 1) * 128],
                    scalar=1.0,
                    in1=id_t[:],
                    op0=Alu.bypass,
                    op1=Alu.mult,
                    accum_out=acc_hm[:, g01 : g01 + 1],
                )
                tile.add_dep_helper(
                    dq_inst.ins, ridu_inst.ins, sync=False,
                    reason="defer hm2 diag past argmax head",
                )
            for b in range(BL):
                dht = scr.tile([128, 128], f32, name=f"dht{b}", tag="dsc")
                dh_inst = nc.vector.scalar_tensor_tensor(
                    out=dht[:],
                    in0=psum_ht[:, b * 128 : (b + 1) * 128],
                    scalar=1.0,
                    in1=id_t[:],
                    op0=Alu.bypass,
                    op1=Alu.mult,
                    accum_out=acc_ht[:, b : b + 1],
                )
                tile.add_dep_helper(
                    dh_inst.ins, ridu_inst.ins, sync=False,
                    reason="defer ht diag past argmax head",
                )

            # ---- y from the gathered rows ----
            ysc = scr.tile([PL, 128], f32, name="ysc", tag="xysc")
            yf = sb.tile([PL, 1], f32, name="yf")
            nc.vector.scalar_tensor_tensor(
                out=ysc[:], in0=gath[:], scalar=Mv[:, 0:1], in1=io_t[0:PL, :],
                op0=Alu.is_equal, op1=Alu.mult, accum_out=yf[:],
            )

            # ---- heat loss: combine per-partition pieces, partition-sum, out ----
            ucomb = sb.tile([128, G], f32, name="ucomb")
            for b in range(BL):
                nc.vector.tensor_tensor(
                    out=ucomb[:, b * S : (b + 1) * S],
                    in0=acc_hm[:, b * S : (b + 1) * S],
                    in1=acc_ht[:, b : b + 1].to_broadcast([128, S]),
                    op=Alu.add,
                )
            acc_fin = sb.tile([128, G], f32, name="acc_fin")
            nc.vector.scalar_tensor_tensor(
                out=acc_fin[:], in0=diag[:], scalar=-2.0, in1=ucomb[:],
                op0=Alu.mult, op1=Alu.add,
            )
            psum_out = ps.tile([1, 2 * G], f32, name="psum_out", tag="fin")
            nc.tensor.matmul(
                out=psum_out[:, 0:G], lhsT=on_t[:], rhs=acc_fin[:],
                start=True, stop=False, skip_group_check=True,
            )

            # ---- label loss ----
            cdiff = sb.tile([PL, 7], f32, name="cdiff")
            nc.vector.tensor_tensor(
                out=cdiff[:], in0=pred9[:, 0:7], in1=lblr[:, 0:7], op=Alu.subtract
            )
            csc = sb.tile([PL, 7], f32, name="csc")
            cls = sb.tile([PL, 1], f32, name="cls")
            nc.scalar.activation(
                out=csc[:], in_=cdiff[:], func=Act.Square, accum_out=cls[:]
            )
            conf = sb.tile([PL, 1], f32, name="conf")
            nc.scalar.activation(
                out=conf[:], in_=Mv[:], func=Act.Square, bias=1.0, scale=-1.0
            )
            t1 = sb.tile([PL, 1], f32, name="t1")
            nc.vector.tensor_tensor(t1[:], lblr[:, 9:10], lblr[:, 7:8], Alu.add)
            t3 = sb.tile([PL, 1], f32, name="t3")
            nc.vector.tensor_tensor(t3[:], lblr[:, 10:11], lblr[:, 8:9], Alu.add)
            gmin = sb.tile([PL, 1], f32, name="gmin")
            nc.vector.tensor_tensor(gmin[:], lblr[:, 9:10], lblr[:, 10:11], Alu.min)
            gmax = sb.tile([PL, 1], f32, name="gmax")
            nc.vector.tensor_tensor(gmax[:], lblr[:, 9:10], lblr[:, 10:11], Alu.max)
            c1 = sb.tile([PL, 1], f32, name="c1")
            nc.vector.tensor_scalar(c1[:], gmin[:], 0.0, None, Alu.is_gt)
            c2t = sb.tile([PL, 1], f32, name="c2t")
            nc.vector.tensor_scalar(c2t[:], gmax[:], float(H), None, Alu.is_lt)
            vv = sb.tile([PL, 1], f32, name="vv")
            nc.vector.tensor_tensor(vv[:], c1[:], c2t[:], Alu.mult)

            # fused stt tail: u = (xf+p7)-t1; w1 = u*u + cls;
            # v = (yf+p8)-t3; w2 = v*v + w1; perkp = (w2+conf)*vv
            u = sb.tile([PL, 1], f32, name="u")
            nc.vector.scalar_tensor_tensor(
                out=u[:], in0=xf[:], scalar=pred9[:, 7:8], in1=t1[:],
                op0=Alu.add, op1=Alu.subtract,
            )
            w1 = sb.tile([PL, 1], f32, name="w1")
            nc.vector.scalar_tensor_tensor(
                out=w1[:], in0=u[:], scalar=u[:, 0:1], in1=cls[:],
                op0=Alu.mult, op1=Alu.add,
            )
            v = sb.tile([PL, 1], f32, name="v")
            nc.vector.scalar_tensor_tensor(
                out=v[:], in0=yf[:], scalar=pred9[:, 8:9], in1=t3[:],
                op0=Alu.add, op1=Alu.subtract,
            )
            w2 = sb.tile([PL, 1], f32, name="w2")
            nc.vector.scalar_tensor_tensor(
                out=w2[:], in0=v[:], scalar=v[:, 0:1], in1=w1[:],
                op0=Alu.mult, op1=Alu.add,
            )
            perkp = sb.tile([PL, 1], f32, name="perkp")
            nc.vector.scalar_tensor_tensor(
                out=perkp[:], in0=w2[:], scalar=conf[:, 0:1], in1=vv[:],
                op0=Alu.add, op1=Alu.mult,
            )
            nc.tensor.matmul(
                out=psum_out[:, G : 2 * G], lhsT=perkp[:], rhs=km_t[:],
                start=False, stop=True, skip_group_check=True,
            )
            out_row = sb.tile([1, 2 * G], f32, name="out_row")
            nc.vector.tensor_copy(out=out_row[:], in_=psum_out[:])
            nc.sync.dma_start(out_all, out_row[:])

        if loop_n > 1:
            # on-device timing loop: each iteration is separated by the
            # For_i back-edge barrier, so wall time ~= N * (span + ~2us)
            with tc.For_i(0, loop_n, 1):
                emit()
        else:
            for _ in range(reps):
                emit()

    nc.compile()
    return nc


def _get_nc(reps=1, loop_n=1):
    key = f"nc{reps}_{loop_n}"
    if key not in _CACHE:
        _CACHE[key] = _build_module(reps, loop_n)
    return _CACHE[key]


def _in_maps(combined_preds, heatmaps, labels):
    cp = np.ascontiguousarray(combined_preds, dtype=np.float32)
    hmr = np.ascontiguousarray(heatmaps, dtype=np.float32)
    lb = np.ascontiguousarray(labels, dtype=np.float32)
    maps = []
    for i in range(NCORES):
        b0 = BL * i
        maps.append(
            {
                "cp": np.ascontiguousarray(cp[b0 : b0 + BL]),
                "hmr": np.ascontiguousarray(hmr[b0 : b0 + BL]),
                "lbl": np.ascontiguousarray(lb[b0 : b0 + BL]),
            }
        )
    return maps


def run(combined_preds, heatmaps, labels, trace=False):
    """Run on hardware; returns ((heat, label), BassKernelResults)."""
    from concourse import bass_utils

    nc = _get_nc()
    res = bass_utils.run_bass_kernel_spmd(
        nc,
        _in_maps(combined_preds, heatmaps, labels),
        core_ids=list(range(NCORES)),
        trace=trace,
    )
    heat = np.concatenate(
        [res.results[i]["out_all"][:, 0:G].reshape(BL, S) for i in range(NCORES)],
        axis=0,
    )
    lab = np.concatenate(
        [res.results[i]["out_all"][:, G : 2 * G].reshape(BL, S) for i in range(NCORES)],
        axis=0,
    )
    return (heat, lab), res


def kernel(combined_preds, heatmaps, labels):
    (heat, lab), _ = run(combined_preds, heatmaps, labels)
    return heat, lab

